# revision 1
# baseline (speedup 1.0000x reference)
"""nn_AgentEncoder on 8 Trainium2 NeuronCores.

Data-parallel over batch (32 batches/core, 16384 agent rows/core). The
history-CNN encoder + valid-gating + type-embedding run on device as a
Bass/Tile kernel; the tiny ego state-attention branch (256x6 tokens) runs
on host and overwrites agent slot 0.

Device kernel layout (per core, R = 16384 rows):
  - raw sequences loaded row-major (rows on partitions, p-major row order
    r = p*16+g so every partition reads one contiguous DRAM run),
  - 9x20 feature map built as a [128, 16, 256] bf16 row-major tile
    (col j = l*9+c; col 192..195 = onehot(category); col 196 = 1-valid),
  - DMA-xbar transposes flip 128-col halves into X0/X1 [128, 512] bf16
    (features on partitions, rows free),
  - 3 conv layers = dense block matmuls (K chunked at j=128 / by position,
    M chunked by output position) with f32 PSUM and fused bias+ReLU
    copies back to bf16 SBUF,
  - mean over positions + transpose back to row-major via accumulating
    matmuls against I/3; type_emb added via onehot @ temb matmul, with a
    correction row that cancels the all-zero-feature response for
    invalid agents,
  - [128 rows, 512] f32 stores.
"""
import math
import os
import time
import numpy as np
import ml_dtypes

B, A, T = 256, 512, 21
DIM = 128
SC = 6
NHEAD, HD = 4, DIM // 4
NCORES = 8
RPC = (B // NCORES) * A        # rows per core = 16384
G = 16
FT_ROWS = 128 * G
NT_ROWS = 512
PI = math.pi

BF16 = ml_dtypes.bfloat16
XBAR = True  # dev flag: False replaces xbar transposes with plain copies
STAGE = int(os.environ.get('KSTAGE', '9'))

WEIGHT_SHAPES = {
    'w1a': (128, 128), 'w1b': (128, 128), 'w1c': (52, 128), 'w1d': (52, 64),
    'w2a': (128, 128), 'w2b': (128, 128), 'w2c': (128, 128),
    'w2d': (64, 128), 'w2e': (64, 64),
    'w3a': (128, 128), 'w3b': (128, 128), 'w3c': (128, 128),
    'w3d': (128, 128), 'w3e': (64, 128),
    'b1ab': (128, 1), 'b1c': (64, 1), 'b2ab': (128, 1), 'b2c': (64, 1),
    'b3': (128, 1), 'i3': (128, 128), 'temb': (69, 128),
}
BF16_KEYS = {'w1a', 'w1b', 'w1c', 'w1d', 'w2a', 'w2b', 'w2c', 'w2d', 'w2e',
             'w3a', 'w3b', 'w3c', 'w3d', 'w3e', 'i3', 'temb'}


def _pack_weights(conv1_w, conv1_b, conv2_w, conv2_b, conv3_w, conv3_b,
                  type_emb):
    W1 = np.zeros((180, 320), np.float32)
    for k in range(3):
        for p in range(10):
            l = 2 * p + k
            if l <= 19:
                W1[l * 9:l * 9 + 9, p * 32:p * 32 + 32] += conv1_w[:, :, k].T
    W2 = np.zeros((320, 320), np.float32)
    for k in range(3):
        for p in range(5):
            l = 2 * p + k
            if l <= 9:
                W2[l * 32:l * 32 + 32, p * 64:p * 64 + 64] += conv2_w[:, :, k].T
    W3 = np.zeros((320, 384), np.float32)
    for k in range(3):
        for p in range(3):
            l = 2 * p + k - 1
            if 0 <= l <= 4:
                W3[l * 64:l * 64 + 64, p * 128:p * 128 + 128] += \
                    conv3_w[:, :, k].T

    d = {}
    d['w1a'] = W1[:128, :128]
    d['w1b'] = W1[:128, 128:256]
    d['w1c'] = W1[128:, 128:256]
    d['w1d'] = W1[128:, 256:]
    d['w2a'] = W2[:128, :128]
    d['w2b'] = W2[128:256, :128]
    d['w2c'] = W2[128:256, 128:256]
    d['w2d'] = W2[256:, 128:256]
    d['w2e'] = W2[256:, 256:]
    d['w3a'] = W3[:128, :128]
    d['w3b'] = W3[:128, 128:256]
    d['w3c'] = W3[128:256, 128:256]
    d['w3d'] = W3[128:256, 256:]
    d['w3e'] = W3[256:, 256:]
    d = {k: np.ascontiguousarray(v, dtype=BF16) for k, v in d.items()}

    d['b1ab'] = np.tile(conv1_b, 4).reshape(128, 1).astype(np.float32)
    d['b1c'] = np.tile(conv1_b, 2).reshape(64, 1).astype(np.float32)
    d['b2ab'] = np.tile(conv2_b, 2).reshape(128, 1).astype(np.float32)
    d['b2c'] = conv2_b.reshape(64, 1).astype(np.float32)
    d['b3'] = conv3_b.reshape(128, 1).astype(np.float32)
    d['i3'] = np.ascontiguousarray(np.eye(128, dtype=np.float32) / 3.0,
                                   dtype=BF16)
    # correction row: cancels the conv chain's all-zero-feature response
    s1z = np.maximum(np.concatenate([conv1_b] * 10), 0.0)
    s2z = np.maximum(s1z @ W2 + np.concatenate([conv2_b] * 5), 0.0)
    h3z = s2z @ W3
    Cz = sum(np.maximum(h3z[p * 128:(p + 1) * 128] + conv3_b, 0.0)
             for p in range(3)) / 3.0
    tpad = np.zeros((69, 128), np.float32)
    tpad[64:68] = type_emb
    tpad[68] = -Cz
    d['temb'] = np.ascontiguousarray(tpad, dtype=BF16)
    return d


def _emit(tc, y, ins, R):
    import concourse.bass as bass  # noqa: F401
    from concourse import mybir

    nc = tc.nc
    f32 = mybir.dt.float32
    bf16 = mybir.dt.bfloat16
    Alu = mybir.AluOpType
    Act = mybir.ActivationFunctionType
    X = mybir.AxisListType.X
    n_ft = R // FT_ROWS

    with tc.tile_pool(name="consts", bufs=1) as cpool:
        wt = {}
        WMODE = int(os.environ.get('KW', '1'))
        for k, shp in WEIGHT_SHAPES.items():
            t = cpool.tile(list(shp), bf16 if k in BF16_KEYS else f32, tag=k)
            if WMODE:
                nc.sync.dma_start(t[:], ins[k])
            wt[k] = t

        with (
            tc.tile_pool(name="raw", bufs=2) as raw,
            tc.tile_pool(name="feat", bufs=2) as feat,
            tc.tile_pool(name="scr", bufs=3) as scr,
            tc.tile_pool(name="xmat", bufs=2) as xmat,
            tc.tile_pool(name="smat", bufs=2) as smat,
            tc.tile_pool(name="psum", bufs=1, space="PSUM") as psum,
        ):
            def stage_a_parts(ft):
                """loads + feature build + transposes for tile ft, emitted as
                4 chunks so the driver can interleave them between the
                previous tile's compute N-tiles."""
                rows = slice(ft * FT_ROWS, (ft + 1) * FT_ROWS)
                st = {}

                def part0():
                    KP = int(os.environ.get('KPARTS', '31'))
                    POS = raw.tile([128, G, 42], f32, tag="pos")
                    VEL = raw.tile([128, G, 42], f32, tag="vel")
                    SHP = raw.tile([128, G, 42], f32, tag="shp")
                    HD = raw.tile([128, G, 21], f32, tag="hd")
                    MSK = raw.tile([128, G, 21], mybir.dt.uint8, tag="msk")
                    CAT = raw.tile([128, G], f32, tag="cat")
                    if KP & 1:
                        nc.sync.dma_start(POS[:], ins['pos'][rows].rearrange(
                            "(p g) k -> p g k", g=G))
                    if KP & 16:
                        nc.sync.dma_start(VEL[:], ins['vel'][rows].rearrange(
                            "(p g) k -> p g k", g=G))
                        nc.sync.dma_start(SHP[:], ins['shp'][rows].rearrange(
                            "(p g) k -> p g k", g=G))
                        nc.sync.dma_start(HD[:], ins['hd'][rows].rearrange(
                            "(p g) t -> p g t", g=G))
                    if KP & 2:
                        nc.sync.dma_start(MSK[:], ins['msk'][rows].rearrange(
                            "(p g) t -> p g t", g=G))
                        nc.sync.dma_start(CAT[:], ins['cat'][rows].rearrange(
                            "(p g) -> p g", g=G))
                    F = feat.tile([128, G, 256], bf16, tag="F")
                    if KP & 8:
                        nc.gpsimd.memset(F[:, :, 180:256], 0.0)
                    M32 = scr.tile([128, G, 21], f32, tag="m32")
                    VMV = scr.tile([128, G, 20], f32, tag="vmv")
                    if KP & 4:
                        nc.vector.tensor_copy(M32[:], MSK[:])
                        nc.vector.tensor_mul(VMV[:], M32[:, :, 1:],
                                             M32[:, :, :20])
                    st.update(POS=POS, VEL=VEL, SHP=SHP, HD=HD, CAT=CAT, F=F,
                              M32=M32, VMV=VMV)

                def part1():
                    if STAGE < 2:
                        return
                    F, VMV = st['F'], st['VMV']
                    for c, RAWT in ((0, st['POS']), (2, st['VEL'])):
                        for comp in range(2):
                            D = scr.tile([128, G, 20], f32, tag="d")
                            nc.vector.tensor_sub(D[:],
                                                 RAWT[:, :, 2 + comp:42:2],
                                                 RAWT[:, :, comp:40:2])
                            cc = c + comp
                            nc.vector.tensor_mul(F[:, :, cc:cc + 172:9], D[:],
                                                 VMV[:])

                def part2():
                    if STAGE < 3:
                        return
                    F, VMV, M32 = st['F'], st['VMV'], st['M32']
                    VARM = scr.tile([128, G, 1], f32, tag="varm")
                    nc.vector.tensor_reduce(VARM[:], M32[:], axis=X,
                                            op=Alu.max)
                    NVA = scr.tile([128, G, 1], f32, tag="nva")
                    nc.vector.tensor_scalar(NVA[:], VARM[:], -1.0, 1.0,
                                            Alu.mult, Alu.add)
                    nc.gpsimd.tensor_copy(F[:, :, 196:197], NVA[:])
                    va_b = VARM[:].to_broadcast([128, G, 20])
                    nva_b = NVA[:].to_broadcast([128, G, 20])
                    DH = scr.tile([128, G, 20], f32, tag="d")
                    nc.vector.tensor_sub(DH[:], st['HD'][:, :, 1:],
                                         st['HD'][:, :, :20])
                    DHM = scr.tile([128, G, 20], f32, tag="dhm")
                    nc.vector.tensor_mul(DHM[:], DH[:], VMV[:])
                    WC = scr.tile([128, G, 20], f32, tag="wc")
                    nc.vector.add_range_wrap(WC[:], DHM[:], PI / 2, PI, 2 * PI)
                    CC = scr.tile([128, G, 20], f32, tag="cc")
                    nc.scalar.activation(CC[:], WC[:], Act.Sin)
                    nc.vector.tensor_sub(F[:, :, 4:4 + 172:9], CC[:], nva_b)
                    WS = scr.tile([128, G, 20], f32, tag="ws")
                    nc.vector.add_range_wrap(WS[:], DHM[:], 0.0, PI, 2 * PI)
                    nc.scalar.activation(F[:, :, 5:5 + 172:9], WS[:], Act.Sin)
                    nc.gpsimd.tensor_tensor(F[:, :, 6:6 + 172:9],
                                            st['SHP'][:, :, 2:42:2], va_b,
                                            op=Alu.mult)
                    nc.gpsimd.tensor_tensor(F[:, :, 7:7 + 172:9],
                                            st['SHP'][:, :, 3:42:2], va_b,
                                            op=Alu.mult)
                    nc.gpsimd.tensor_copy(F[:, :, 8:8 + 172:9], VMV[:])
                    for j in range(4):
                        nc.vector.tensor_scalar(
                            F[:, :, 192 + j:193 + j],
                            st['CAT'][:, :].unsqueeze(2),
                            float(j), None, Alu.is_equal)

                return [part0, part1, part2], st


            def stage_b_tile(ft, n, F):
                if STAGE < 4:
                    if ft == 0 and n == 0:
                        M = smat.tile([128, NT_ROWS], f32, tag="outd")
                        nc.gpsimd.memset(M[:], 0.0)
                        nc.sync.dma_start(
                            y[0:NT_ROWS].rearrange("(p g) d -> p (g d)",
                                                   g=4), M[:])
                    return
                """transposes + conv matmuls + relus + mean/temb + store,
                one N-tile. Transposes split across both HWDGE engines
                (all-on-SP exceeds a walrus xbar-register limit; so do
                long consecutive runs on one engine)."""
                rows = slice(ft * FT_ROWS, (ft + 1) * FT_ROWS)
                if True:
                    X0 = xmat.tile([128, NT_ROWS], bf16, tag="x0")
                    X1 = xmat.tile([128, NT_ROWS], bf16, tag="x1")
                    for gi in range(4):
                        g = 4 * n + gi
                        cs = slice(gi * 128, gi * 128 + 128)
                        if XBAR:
                            # both halves on SP: ACT-dispatched DMAs head-of-
                            # line-block ACT compute in its FIFO (sim: -25%)
                            nc.sync.dma_start_transpose(X0[:, cs],
                                                        F[:, g, 0:128])
                            nc.sync.dma_start_transpose(X1[:, cs],
                                                        F[:, g, 128:256])
                        else:
                            nc.sync.dma_start(X0[:, cs], F[:, g, 0:128])
                            nc.sync.dma_start(X1[:, cs], F[:, g, 128:256])
                    h1a = psum.tile([128, NT_ROWS], f32, tag="h1a")
                    h1b = psum.tile([128, NT_ROWS], f32, tag="h1b")
                    h1c = psum.tile([64, NT_ROWS], f32, tag="h1c")
                    nc.tensor.matmul(h1a[:], wt['w1a'][:], X0[:, :],
                                     start=True, stop=True)
                    nc.tensor.matmul(h1b[:], wt['w1b'][:], X0[:, :],
                                     start=True, stop=False)
                    nc.tensor.matmul(h1b[:], wt['w1c'][:], X1[0:52, :],
                                     start=False, stop=True)
                    nc.tensor.matmul(h1c[:], wt['w1d'][:], X1[0:52, :],
                                     start=True, stop=True)
                    s1a = smat.tile([128, NT_ROWS], bf16, tag="s1a")
                    s1b = smat.tile([128, NT_ROWS], bf16, tag="s1b")
                    s1c = smat.tile([64, NT_ROWS], bf16, tag="s1c")
                    nc.scalar.activation(s1a[:], h1a[:], Act.Relu,
                                         bias=wt['b1ab'][:])
                    nc.scalar.activation(s1b[:], h1b[:], Act.Relu,
                                         bias=wt['b1ab'][:])
                    nc.vector.tensor_scalar(s1c[:], h1c[:], wt['b1c'][:], 0.0,
                                            Alu.add, Alu.max)

                    if STAGE < 5:
                        OUTD = smat.tile([128, NT_ROWS], f32, tag="outd")
                        nc.vector.tensor_copy(OUTD[:], h1a[:])
                        nc.sync.dma_start(
                            y[rows].rearrange("(p g) d -> p g d", g=G)[
                                :, 4 * n:4 * n + 4, :].rearrange(
                                "p a b -> p (a b)"), OUTD[:])
                        return
                    h2a = psum.tile([128, NT_ROWS], f32, tag="h2a")
                    h2b = psum.tile([128, NT_ROWS], f32, tag="h2b")
                    h2c = psum.tile([64, NT_ROWS], f32, tag="h2c")
                    nc.tensor.matmul(h2a[:], wt['w2a'][:], s1a[:],
                                     start=True, stop=False)
                    nc.tensor.matmul(h2a[:], wt['w2b'][:], s1b[:],
                                     start=False, stop=True)
                    nc.tensor.matmul(h2b[:], wt['w2c'][:], s1b[:],
                                     start=True, stop=False)
                    nc.tensor.matmul(h2b[:], wt['w2d'][:], s1c[:],
                                     start=False, stop=True)
                    nc.tensor.matmul(h2c[:], wt['w2e'][:], s1c[:],
                                     start=True, stop=True)
                    s2a = smat.tile([128, NT_ROWS], bf16, tag="s2a")
                    s2b = smat.tile([128, NT_ROWS], bf16, tag="s2b")
                    s2c = smat.tile([64, NT_ROWS], bf16, tag="s2c")
                    nc.scalar.activation(s2a[:], h2a[:], Act.Relu,
                                         bias=wt['b2ab'][:])
                    nc.vector.tensor_scalar(s2b[:], h2b[:], wt['b2ab'][:], 0.0,
                                            Alu.add, Alu.max)
                    nc.scalar.activation(s2c[:], h2c[:], Act.Relu,
                                         bias=wt['b2c'][:])

                    rts = []
                    mm3 = [
                        [('w3a', s2a, slice(0, 128))],
                        [('w3b', s2a, slice(0, 128)),
                         ('w3c', s2b, slice(0, 128))],
                        [('w3d', s2b, slice(0, 128)),
                         ('w3e', s2c, slice(0, 64))],
                    ]
                    for p in range(3):
                        h3 = psum.tile([128, NT_ROWS], f32, tag="h3")
                        terms = mm3[p]
                        for i, (wk, src, ks) in enumerate(terms):
                            nc.tensor.matmul(h3[:], wt[wk][:], src[ks, :],
                                             start=(i == 0),
                                             stop=(i == len(terms) - 1))
                        t_p = smat.tile([128, NT_ROWS], bf16, tag=f"t{p}")
                        if p == 1:
                            nc.vector.tensor_scalar(t_p[:], h3[:],
                                                    wt['b3'][:], 0.0,
                                                    Alu.add, Alu.max)
                        else:
                            nc.scalar.activation(t_p[:], h3[:], Act.Relu,
                                                 bias=wt['b3'][:])
                        rts.append(t_p)

                    enc = psum.tile([128, 4, 128], f32, tag="enc")
                    for gi in range(4):
                        cs = slice(gi * 128, gi * 128 + 128)
                        nc.tensor.matmul(enc[:, gi, :], rts[0][:, cs],
                                         wt['i3'][:], start=True, stop=False)
                        nc.tensor.matmul(enc[:, gi, :], rts[1][:, cs],
                                         wt['i3'][:], start=False, stop=False)
                        nc.tensor.matmul(enc[:, gi, :], rts[2][:, cs],
                                         wt['i3'][:], start=False, stop=False)
                        nc.tensor.matmul(enc[:, gi, :], X1[64:69, cs],
                                         wt['temb'][64:69, :],
                                         start=False, stop=True)

                    OUT = smat.tile([128, 4, 128], f32, tag="out")
                    nc.vector.tensor_copy(OUT[:], enc[:])
                    nc.sync.dma_start(
                        y[rows].rearrange("(p g) d -> p g d",
                                          g=G)[:, 4 * n:4 * n + 4, :], OUT[:])

            # 2-stage software pipeline, interleaved at N-tile granularity:
            # feature-build chunks of tile ft slot between the compute tiles
            # of ft-1 so relu copies keep priority on DVE/ACT while features
            # fill the gaps.
            NT = FT_ROWS // NT_ROWS
            for ft in range(n_ft):
                parts, st = stage_a_parts(ft)
                for pf in parts:
                    pf()
                for n in range(NT):
                    stage_b_tile(ft, n, st['F'])


_BUILT = None


def _build():
    global _BUILT
    if _BUILT is not None:
        return _BUILT
    import concourse.tile as tile
    from concourse import bacc, mybir

    nc = bacc.Bacc("TRN2", target_bir_lowering=False, debug=False,
                   num_devices=NCORES)
    ins = {}
    ins['pos'] = nc.dram_tensor("inpos", [RPC, 42], mybir.dt.float32,
                                kind="ExternalInput").ap()
    ins['vel'] = nc.dram_tensor("invel", [RPC, 42], mybir.dt.float32,
                                kind="ExternalInput").ap()
    ins['shp'] = nc.dram_tensor("inshp", [RPC, 42], mybir.dt.float32,
                                kind="ExternalInput").ap()
    ins['hd'] = nc.dram_tensor("inhd", [RPC, 21], mybir.dt.float32,
                               kind="ExternalInput").ap()
    ins['msk'] = nc.dram_tensor("inmsk", [RPC, 21], mybir.dt.uint8,
                                kind="ExternalInput").ap()
    ins['cat'] = nc.dram_tensor("incat", [RPC], mybir.dt.float32,
                                kind="ExternalInput").ap()
    for k, shp in WEIGHT_SHAPES.items():
        dt = mybir.dt.bfloat16 if k in BF16_KEYS else mybir.dt.float32
        ins[k] = nc.dram_tensor("w_" + k, list(shp), dt,
                                kind="ExternalInput").ap()
    y = nc.dram_tensor("y", [RPC, DIM], mybir.dt.float32,
                       kind="ExternalOutput").ap()

    with tile.TileContext(nc) as tc:
        _emit(tc, y, ins, RPC)
    nc.finalize()   # register allocation etc. — walrus rejects unfinalized BIR
    _BUILT = nc
    return nc


def _host_ego(current_state, query, se_w, se_b, pos_embed, in_proj_w,
              in_proj_b, out_proj_w, out_proj_b):
    ego = current_state[:, :SC].astype(np.float32)
    x_embed = ego[:, :, None] * se_w[None] + se_b[None] + pos_embed
    Wq, Wk, Wv = in_proj_w[:DIM], in_proj_w[DIM:2 * DIM], in_proj_w[2 * DIM:]
    bq, bk, bv = in_proj_b[:DIM], in_proj_b[DIM:2 * DIM], in_proj_b[2 * DIM:]
    q = (query[0, 0] @ Wq.T + bq).reshape(NHEAD, HD)
    k = (x_embed @ Wk.T + bk).reshape(B, SC, NHEAD, HD)
    v = (x_embed @ Wv.T + bv).reshape(B, SC, NHEAD, HD)
    scores = np.einsum('hd,bshd->bhs', q, k, optimize=True) / np.sqrt(HD)
    scores -= scores.max(axis=-1, keepdims=True)
    e = np.exp(scores)
    attn = e / e.sum(axis=-1, keepdims=True)
    o = np.einsum('bhs,bshd->bhd', attn, v, optimize=True).reshape(B, DIM)
    return o @ out_proj_w.T + out_proj_b


LAST_EXEC_NS = None
_RUNNER = None


def _get_runner():
    """Cached jitted shard_map runner over the 8 cores (mirrors
    bass2jax.run_bass_via_pjrt's multi-core path, but reusable across
    calls so the executable compiles once and inputs can stay device-
    resident during timing)."""
    global _RUNNER
    if _RUNNER is not None:
        return _RUNNER
    import jax
    from jax.sharding import Mesh, PartitionSpec
    from jax.experimental.shard_map import shard_map
    from concourse import bass2jax, mybir

    bass2jax.install_neuronx_cc_hook()
    nc = _build()
    partition_name = (nc.partition_id_tensor.name
                      if nc.partition_id_tensor else None)
    in_names, out_names, out_avals, zero_outs = [], [], [], []
    for alloc in nc.m.functions[0].allocations:
        if not isinstance(alloc, mybir.MemoryLocationSet):
            continue
        name = alloc.memorylocations[0].name
        if alloc.kind == "ExternalInput":
            if name != partition_name:
                in_names.append(name)
        elif alloc.kind == "ExternalOutput":
            shape = tuple(alloc.tensor_shape)
            dtype = mybir.dt.np(alloc.dtype)
            out_names.append(name)
            out_avals.append(jax.core.ShapedArray(shape, dtype))
            zero_outs.append((shape, dtype))
    n_params = len(in_names)
    all_names = in_names + out_names
    if partition_name is not None:
        all_names.append(partition_name)

    def _body(*args):
        operands = list(args)
        if partition_name is not None:
            operands.append(bass2jax.partition_id_tensor())
        return tuple(bass2jax._bass_exec_p.bind(
            *operands,
            out_avals=tuple(out_avals),
            in_names=tuple(all_names),
            out_names=tuple(out_names),
            lowering_input_output_aliases=(),
            sim_require_finite=True,
            sim_require_nnan=True,
            nc=nc,
        ))

    devices = jax.devices()[:NCORES]
    mesh = Mesh(np.asarray(devices), ("core",))
    n_outs = len(out_names)
    sharded = jax.jit(
        shard_map(_body, mesh=mesh,
                  in_specs=(PartitionSpec("core"),) * (n_params + n_outs),
                  out_specs=(PartitionSpec("core"),) * n_outs,
                  check_rep=False),
        donate_argnums=tuple(range(n_params, n_params + n_outs)),
        keep_unused=True,
    )
    _RUNNER = dict(sharded=sharded, in_names=in_names, out_names=out_names,
                   out_avals=out_avals, zero_outs=zero_outs, mesh=mesh)
    return _RUNNER


def _run_device(in_maps, iters=1):
    """Execute on the 8 cores; returns (list-of-per-core-output-dicts,
    best_exec_ns measured over `iters` runs with device-resident inputs)."""
    import jax
    from jax.sharding import NamedSharding, PartitionSpec

    r = _get_runner()
    sharded = r['sharded']
    spec = NamedSharding(r['mesh'], PartitionSpec("core"))
    concat_in = [
        jax.device_put(
            np.concatenate([np.asarray(m[name]) for m in in_maps], axis=0)
            if np.asarray(in_maps[0][name]).ndim > 0 else
            np.concatenate([np.asarray(m[name]).reshape(1) for m in in_maps]),
            spec)
        for name in r['in_names']
    ]
    jax.block_until_ready(concat_in)
    best = None
    out_arrs = None
    for _ in range(max(1, iters)):
        zeros = [
            jax.device_put(np.zeros((NCORES * s[0], *s[1:]), d), spec)
            for (s, d) in r['zero_outs']
        ]
        jax.block_until_ready(zeros)
        t0 = time.perf_counter()
        out_arrs = sharded(*concat_in, *zeros)
        jax.block_until_ready(out_arrs)
        dt = time.perf_counter() - t0
        best = dt if best is None else min(best, dt)
    outs = []
    np_outs = [np.asarray(a) for a in out_arrs]
    for c in range(NCORES):
        outs.append({
            name: np_outs[i].reshape(NCORES, *r['out_avals'][i].shape)[c]
            for i, name in enumerate(r['out_names'])
        })
    return outs, int(best * 1e9)


def kernel(**inputs):
    global LAST_EXEC_NS
    position = np.asarray(inputs['position'], np.float32)
    heading = np.asarray(inputs['heading'], np.float32)
    velocity = np.asarray(inputs['velocity'], np.float32)
    shape = np.asarray(inputs['shape'], np.float32)
    current_state = np.asarray(inputs['current_state'], np.float32)
    category = np.asarray(inputs['category'])
    valid_mask = np.asarray(inputs['valid_mask']).astype(np.uint8)

    w = _pack_weights(
        np.asarray(inputs['conv1_w'], np.float32),
        np.asarray(inputs['conv1_b'], np.float32),
        np.asarray(inputs['conv2_w'], np.float32),
        np.asarray(inputs['conv2_b'], np.float32),
        np.asarray(inputs['conv3_w'], np.float32),
        np.asarray(inputs['conv3_b'], np.float32),
        np.asarray(inputs['type_emb'], np.float32))

    bpc = B // NCORES
    cat_f = category.astype(np.float32)
    in_maps = []
    for c in range(NCORES):
        bs = slice(c * bpc, (c + 1) * bpc)
        m = {
            'inpos': position[bs].reshape(RPC, 42),
            'invel': velocity[bs].reshape(RPC, 42),
            'inshp': shape[bs].reshape(RPC, 42),
            'inhd': heading[bs].reshape(RPC, 21),
            'inmsk': valid_mask[bs].reshape(RPC, 21),
            'incat': cat_f[bs].reshape(RPC),
        }
        for k in WEIGHT_SHAPES:
            m['w_' + k] = w[k]
        in_maps.append(m)

    # 2 timed executes per call: inputs stay device-resident, so the second
    # iteration costs only the execute — doubles timing samples against the
    # noisy axon-tunnel RPC floor
    res, LAST_EXEC_NS = _run_device(in_maps, iters=2)

    out = np.concatenate(
        [r['y'].reshape(bpc, A, DIM) for r in res], axis=0)

    x_ego = _host_ego(current_state, np.asarray(inputs['query'], np.float32),
                      np.asarray(inputs['se_w'], np.float32),
                      np.asarray(inputs['se_b'], np.float32),
                      np.asarray(inputs['pos_embed'], np.float32),
                      np.asarray(inputs['in_proj_w'], np.float32),
                      np.asarray(inputs['in_proj_b'], np.float32),
                      np.asarray(inputs['out_proj_w'], np.float32),
                      np.asarray(inputs['out_proj_b'], np.float32))
    temb = np.asarray(inputs['type_emb'], np.float32)
    out[:, 0, :] = x_ego + temb[category[:, 0]]
    return np.ascontiguousarray(out, dtype=np.float32)

